# revision 1
# baseline (speedup 1.0000x reference)
"""LAINet forward (nn_LAINetOriginal) on 8 NeuronCores.

Sharding: windows (1000) split 8 x 125 across cores, each core also
recomputes a 37-window reflect-mapped halo so the Conv2d smoother needs no
cross-core communication. BatchNorm stats are over the batch axis, which is
fully local under window sharding, so numerics match the reference exactly
(fp32 everywhere).

Self-contained: only numpy + jax. kernel(**inputs) takes full inputs and
returns (out_base, out_smooth), both [64, 7, 1000, 2] float32.
"""
import numpy as np

B = 64
INPUT_DIM = 500000
WIN = 500
N_WIN = 1000
HID = 30
ANC = 7
KS = 75
EPS = 1e-5
NCORES = 8
OWN = N_WIN // NCORES          # 125
HALO = KS // 2                 # 37
LWIN = OWN + 2 * HALO          # 199


def _core_windows(k):
    idx = []
    for i in range(OWN * k - HALO, OWN * (k + 1) + HALO):
        if i < 0:
            i = -i
        elif i > N_WIN - 1:
            i = 2 * (N_WIN - 1) - i
        idx.append(i)
    return np.asarray(idx, dtype=np.int32)


def _core_fn_jax(xw, W1k, b1k, W2k, b2k, conv_w, conv_b):
    import jax.numpy as jnp
    from jax import lax, nn
    # xw: [B, LWIN, WIN, 2] already scaled to [-1, 1]
    h = jnp.einsum('bnwc,nwh->bnhc', xw, W1k) + b1k[None, :, :, None]
    h = nn.relu(h)
    mean = jnp.mean(h, axis=0, keepdims=True)
    var = jnp.var(h, axis=0, keepdims=True)
    h = (h - mean) * lax.rsqrt(var + EPS)
    o = jnp.einsum('bnhc,nha->bnac', h, W2k) + b2k[None, :, :, None]
    o = jnp.transpose(o, (0, 2, 1, 3))            # [B, A, LWIN, 2]
    p = nn.softmax(o, axis=1)
    # halo windows already implement the reflect pad along the window dim;
    # only the 2-wide channel dim still needs its (1,1) reflect pad.
    pp = jnp.pad(p, ((0, 0), (0, 0), (0, 0), (1, 1)), mode='reflect')
    out = lax.conv_general_dilated(
        pp, conv_w, window_strides=(1, 1), padding='VALID',
        dimension_numbers=('NCHW', 'OIHW', 'NCHW'))
    out = out + conv_b[None, :, None, None]
    ob = o[:, :, HALO:HALO + OWN, :]
    os_ = out[:, :, :, 0:2]                        # [B, A, OWN, 2]
    return ob, os_


def _core_fn_np(xw, W1k, b1k, W2k, b2k, conv_w, conv_b):
    h = np.einsum('bnwc,nwh->bnhc', xw, W1k) + b1k[None, :, :, None]
    np.maximum(h, 0.0, out=h)
    mean = h.mean(axis=0, keepdims=True)
    var = h.var(axis=0, keepdims=True)
    h = (h - mean) / np.sqrt(var + EPS)
    o = np.einsum('bnhc,nha->bnac', h, W2k) + b2k[None, :, :, None]
    o = np.transpose(o, (0, 2, 1, 3)).copy()       # [B, A, LWIN, 2]
    e = np.exp(o - o.max(axis=1, keepdims=True))
    p = e / e.sum(axis=1, keepdims=True)
    pp = np.pad(p, ((0, 0), (0, 0), (0, 0), (1, 1)), mode='reflect')
    # conv NCHW/OIHW valid: out[b,o,y,x] = sum_{i,t,w} cw[o,i,t,w] pp[b,i,y+t,x+w]
    Bn, Ci, Hh, Ww = pp.shape
    out = np.zeros((Bn, ANC, Hh - KS + 1, Ww - 1), np.float32)
    for t in range(KS):
        for w in range(2):
            # [o, i] x [b, i, y, x] -> [b, o, y, x]
            out += np.einsum(
                'oi,biyx->boyx', conv_w[:, :, t, w],
                pp[:, :, t:t + Hh - KS + 1, w:w + Ww - 1])
    out += conv_b[None, :, None, None]
    return o[:, :, HALO:HALO + OWN, :], out[:, :, :, 0:2]


def _prep_shards(x, W1, b1, W2, b2):
    xs = (np.asarray(x, np.float32) - 0.5) * 2.0
    xr = xs.reshape(B, N_WIN, WIN, 2)
    shards = []
    for k in range(NCORES):
        idx = _core_windows(k)
        shards.append((np.ascontiguousarray(xr[:, idx]),
                       np.ascontiguousarray(W1[idx]),
                       np.ascontiguousarray(b1[idx]),
                       np.ascontiguousarray(W2[idx]),
                       np.ascontiguousarray(b2[idx])))
    return shards


def kernel(x, W1, b1, W2, b2, conv_w, conv_b):
    W1 = np.asarray(W1, np.float32)
    b1 = np.asarray(b1, np.float32)
    W2 = np.asarray(W2, np.float32)
    b2 = np.asarray(b2, np.float32)
    conv_w = np.asarray(conv_w, np.float32)
    conv_b = np.asarray(conv_b, np.float32)
    shards = _prep_shards(x, W1, b1, W2, b2)

    ob = np.empty((B, ANC, N_WIN, 2), np.float32)
    os_ = np.empty((B, ANC, N_WIN, 2), np.float32)
    try:
        import jax
        devs = jax.devices()
        assert len(devs) >= NCORES
        fn = jax.jit(_core_fn_jax)
        outs = []
        for k in range(NCORES):
            dev = devs[k]
            args = jax.device_put(shards[k], dev) + (
                jax.device_put(conv_w, dev), jax.device_put(conv_b, dev))
            outs.append(fn(*args))
        for k in range(NCORES):
            obk, osk = outs[k]
            ob[:, :, OWN * k:OWN * (k + 1)] = np.asarray(obk)
            os_[:, :, OWN * k:OWN * (k + 1)] = np.asarray(osk)
        return ob, os_
    except Exception:
        for k in range(NCORES):
            obk, osk = _core_fn_np(*shards[k], conv_w, conv_b)
            ob[:, :, OWN * k:OWN * (k + 1)] = obk
            os_[:, :, OWN * k:OWN * (k + 1)] = osk
        return ob, os_

